# revision 13
# baseline (speedup 1.0000x reference)
"""ProbSparse self-attention (Informer-style) TRN2 Bass kernel.

Full inputs in, full output out. Sharding: 8 cores = (2 batches) x (4 head
groups of 4 heads). Each core computes Q/K/V projections for its 4 heads,
exact top-38 sparse attention, and a partial output projection; the host sums
the 4 partials per batch and adds the (constant) bias corrections.

Per (head, q-tile) pipeline on device:
  PE:   S = Q_h q-tile @ K_h^T            (4x [128,512] psum matmuls, K=64)
  ACT:  s_sb = copy(S_psum)
  DVE:  exact top-38 threshold per row via chunked max8/match_replace:
        16x per-chunk top-8 -> candidates; leftovers R; top-8(R) supplement
        (covers up to 8 "hidden" values; P(fail) ~1e-8/row); 5 rounds of
        max8/match_replace over the 136 candidates -> sorted top-40 (mvals)
  ACT:  p = exp(s/8); Z = sum(exp(top38/8)) with exact-tie handling
  GPS:  mask = (s >= thr) * (1/Z); pm = mask * p          (normalized probs)
  PE:   16x transpose(pm chunk) -> psum (4 per bank); ACT copies -> pmT
  PE:   ctx^T = sum_k V_chunk-as-lhsT @ pmT chunk, two heads col-tiled into
        one [128,128] psum bank (tile_position concurrency)
Then out_partial = ctx^T.T @ Wo_slice^T via PE per l-tile (inline), DMA'd out.
"""

import numpy as np

B = 2
L = 2048
DM = 1024
H = 16
DH = 64          # head dim
HPC = 4          # heads per core
U = 38           # top-k (min(L, int(5*log(2048))))
NCH = 16         # key chunks of 128 per row
NEG = -3.0e38
NCORES = 8

_CACHED = {}


def _build_nc():
    import concourse.mybir as mybir
    import concourse.tile as tile
    from concourse import bacc
    from concourse.masks import make_identity

    fp32 = mybir.dt.float32
    Alu = mybir.AluOpType
    Act = mybir.ActivationFunctionType

    nc = bacc.Bacc("TRN2", target_bir_lowering=False, debug=False)

    xT = nc.dram_tensor("xT", [DM, L], fp32, kind="ExternalInput")
    wq = nc.dram_tensor("wq", [DM, HPC * DH], fp32, kind="ExternalInput")
    wk = nc.dram_tensor("wk", [DM, HPC * DH], fp32, kind="ExternalInput")
    wv = nc.dram_tensor("wv", [DM, HPC * DH], fp32, kind="ExternalInput")
    wo = nc.dram_tensor("wo", [HPC * DH, DM], fp32, kind="ExternalInput")
    bq = nc.dram_tensor("bq", [HPC * DH, 1], fp32, kind="ExternalInput")
    bk = nc.dram_tensor("bk", [HPC * DH, 1], fp32, kind="ExternalInput")
    out = nc.dram_tensor("out", [L, DM], fp32, kind="ExternalOutput")

    JW = HPC * DH          # 256 local head-dim width
    NQT = L // 128         # 16 q tiles
    with tile.TileContext(nc) as tc:
        with tc.tile_pool(name="persist", bufs=1) as pp:
            # Persistent SBUF tensors
            qt_sb = [pp.tile([128, L], fp32, tag=f"qt{j}", name=f"qt{j}") for j in range(2)]
            kt_sb = [pp.tile([128, L], fp32, tag=f"kt{j}", name=f"kt{j}") for j in range(2)]
            v_sb = pp.tile([128, NQT * JW], fp32, tag="v", name="v_sb")      # [l-chunk, lt*256 + hh*64 + d]
            ctxT_sb = [pp.tile([128, L], fp32, tag=f"ctxT{j}", name=f"ctxT{j}") for j in range(2)]
            wo_sb = [pp.tile([128, DM], fp32, tag=f"wo{j}", name=f"wo{j}") for j in range(2)]
            ident = pp.tile([128, 128], fp32, tag="ident", name="ident")
            bq_sb = [pp.tile([128, 1], fp32, tag=f"bq{j}", name=f"bq{j}") for j in range(2)]
            bk_sb = [pp.tile([128, 1], fp32, tag=f"bk{j}", name=f"bk{j}") for j in range(2)]

            make_identity(nc, ident[:])
            for j in range(2):
                nc.sync.dma_start(wo_sb[j][:], wo[j * 128:(j + 1) * 128, :])
                nc.sync.dma_start(bq_sb[j][:], bq[j * 128:(j + 1) * 128, :])
                nc.sync.dma_start(bk_sb[j][:], bk[j * 128:(j + 1) * 128, :])

            # ---------------- Projections ----------------
            with tc.tile_pool(name="proj", bufs=1) as xp, \
                 tc.tile_pool(name="projw", bufs=1) as wp, \
                 tc.tile_pool(name="projps", bufs=4, space="PSUM") as pps:
                xt_t = [xp.tile([128, L], fp32, tag=f"xt{cc}", name=f"xt{cc}") for cc in range(8)]
                wq_t = wp.tile([128, 8 * JW], fp32, tag="wqt", name="wq_t")
                wk_t = wp.tile([128, 8 * JW], fp32, tag="wkt", name="wk_t")
                wv_t = wp.tile([128, 8 * JW], fp32, tag="wvt", name="wv_t")
                for cc in range(8):
                    nc.sync.dma_start(xt_t[cc][:], xT[cc * 128:(cc + 1) * 128, :])
                    nc.sync.dma_start(wq_t[:, cc * JW:(cc + 1) * JW], wq[cc * 128:(cc + 1) * 128, :])
                    nc.sync.dma_start(wk_t[:, cc * JW:(cc + 1) * JW], wk[cc * 128:(cc + 1) * 128, :])
                    nc.sync.dma_start(wv_t[:, cc * JW:(cc + 1) * JW], wv[cc * 128:(cc + 1) * 128, :])

                # Q^T, K^T: [256 j, 2048 l] as 2 tiles of [128, 2048]
                for (wt, dst, bias) in ((wq_t, qt_sb, bq_sb), (wk_t, kt_sb, bk_sb)):
                    for jt in range(2):
                        for lc in range(4):
                            ps = pps.tile([128, 512], fp32, tag="projp", name="projp")
                            for cc in range(8):
                                nc.tensor.matmul(
                                    ps[:],
                                    wt[:, cc * JW + jt * 128: cc * JW + (jt + 1) * 128],
                                    xt_t[cc][:, lc * 512:(lc + 1) * 512],
                                    start=(cc == 0), stop=(cc == 7))
                            nc.vector.tensor_scalar(
                                out=dst[jt][:, lc * 512:(lc + 1) * 512], in0=ps[:],
                                scalar1=bias[jt][:, 0:1], scalar2=None, op0=Alu.add)
                # V: [2048 l, 256 j] as v_sb[l%128, (l//128)*256 + j]
                for lt in range(NQT):
                    ps = pps.tile([128, JW], fp32, tag="projv", name="projv")
                    for cc in range(8):
                        nc.tensor.matmul(
                            ps[:],
                            xt_t[cc][:, lt * 128:(lt + 1) * 128],
                            wv_t[:, cc * JW:(cc + 1) * JW],
                            start=(cc == 0), stop=(cc == 7))
                    nc.scalar.copy(v_sb[:, lt * JW:(lt + 1) * JW], ps[:])

            # ---------------- Attention (it-major; Wo inline per l-tile) ----------------
            with tc.tile_pool(name="att", bufs=2) as ap, \
                 tc.tile_pool(name="atts", bufs=4) as sp, \
                 tc.tile_pool(name="fin", bufs=3) as fop, \
                 tc.tile_pool(name="attps", bufs=4, space="PSUM") as sps, \
                 tc.tile_pool(name="attpt", bufs=2, space="PSUM") as tps, \
                 tc.tile_pool(name="attpc", bufs=1, space="PSUM") as cps, \
                 tc.tile_pool(name="attpw", bufs=1, space="PSUM") as wps:

                def head_scores(jt, po, it):
                    """scores for one head -> s_sb (SBUF)"""
                    s_sb = ap.tile([128, L], fp32, tag="s_sb", name="s_sb", bufs=4)
                    for ncb in range(4):
                        s_ps = sps.tile([128, 512], fp32, tag="sps", name="s_ps")
                        nc.tensor.matmul(
                            s_ps[:],
                            qt_sb[jt][po:po + 64, it * 128:(it + 1) * 128],
                            kt_sb[jt][po:po + 64, ncb * 512:(ncb + 1) * 512],
                            start=True, stop=True)
                        nc.scalar.copy(s_sb[:, ncb * 512:(ncb + 1) * 512], s_ps[:])
                    return s_sb

                def head_softmax(s_sb):
                    """top-38 -> normalized masked probs -> pmT"""
                    # exact top-38 threshold
                    cand = sp.tile([128, NCH * 8 + 8], fp32, tag="cand", name="cand")
                    r = ap.tile([128, L], fp32, tag="r", name="r", bufs=3)
                    for c in range(NCH):
                        sl = slice(c * 128, (c + 1) * 128)
                        nc.vector.max(out=cand[:, c * 8:(c + 1) * 8], in_=s_sb[:, sl])
                        nc.vector.match_replace(
                            out=r[:, sl], in_to_replace=cand[:, c * 8:(c + 1) * 8],
                            in_values=s_sb[:, sl], imm_value=NEG)
                    nc.vector.max(out=cand[:, NCH * 8:], in_=r[:])
                    mvals = sp.tile([128, 40], fp32, tag="mvals", name="mvals")
                    scratch = sp.tile([128, NCH * 8 + 8], fp32, tag="scr", name="scratch")
                    cur = cand
                    for rnd in range(5):
                        nc.vector.max(out=mvals[:, rnd * 8:(rnd + 1) * 8], in_=cur[:])
                        if rnd < 4:
                            nc.vector.match_replace(
                                out=scratch[:], in_to_replace=mvals[:, rnd * 8:(rnd + 1) * 8],
                                in_values=cur[:], imm_value=NEG)
                            cur = scratch
                    thr = mvals[:, U - 1:U]

                    # unnormalized exp (independent of Z)
                    p = ap.tile([128, L], fp32, tag="p", name="p", bufs=3)
                    nc.scalar.activation(p[:], s_sb[:], Act.Exp, scale=0.125)

                    # Z = sum(exp(top-38/8)) with exact-tie handling
                    emv = sp.tile([128, 40], fp32, tag="emv", name="emv")
                    nc.scalar.activation(emv[:], mvals[:], Act.Exp, scale=0.125)
                    tie = sp.tile([128, 2], fp32, tag="tie", name="tie")
                    nc.gpsimd.tensor_scalar(out=tie[:], in0=mvals[:, U:],
                                            scalar1=thr, scalar2=None, op0=Alu.is_ge)
                    nc.gpsimd.tensor_tensor(out=emv[:, U:], in0=emv[:, U:],
                                            in1=tie[:], op=Alu.mult)
                    zz = sp.tile([128, 1], fp32, tag="zz", name="zz")
                    nc.vector.reduce_sum(zz[:], emv[:], axis=mybir.AxisListType.X)
                    rz = sp.tile([128, 1], fp32, tag="rz", name="rz")
                    nc.vector.reciprocal(rz[:], zz[:])

                    # masked, 1/Z-scaled probabilities (gpsimd)
                    mask = ap.tile([128, L], fp32, tag="r", name="mask", bufs=3)
                    nc.gpsimd.tensor_scalar(out=mask[:], in0=s_sb[:], scalar1=thr,
                                            scalar2=rz[:, 0:1], op0=Alu.is_ge,
                                            op1=Alu.mult)
                    pm = p
                    nc.gpsimd.tensor_tensor(out=pm[:], in0=mask[:], in1=p[:], op=Alu.mult)

                    # transpose pm (4 transposes per psum bank, 1 copy per bank)
                    pmT = ap.tile([128, L], fp32, tag="pmT", name="pmT", bufs=3)
                    for g in range(4):
                        pt_ps = tps.tile([128, 512], fp32, tag="ptps", name="pt_ps")
                        for cc in range(4):
                            c = g * 4 + cc
                            nc.tensor.transpose(pt_ps[:, cc * 128:(cc + 1) * 128],
                                                pm[:, c * 128:(c + 1) * 128], ident[:])
                        nc.scalar.copy(pmT[:, g * 512:(g + 1) * 512], pt_ps[:])
                    return pmT

                for it in range(NQT):
                    for jt in range(2):
                        # both heads' score matmuls adjacent in the PE stream:
                        # disjoint row-groups (0-63 / 64-127) overlap on HW
                        s_sbs = [head_scores(jt, hp * 64, it) for hp in (0, 1)]
                        pmTs = [head_softmax(s_sbs[hp]) for hp in (0, 1)]
                        # paired AV: two heads col-tiled into one [128,128] psum bank
                        ctx_ps = cps.tile([128, 128], fp32, tag="ctxps", name="ctx_ps")
                        for kc in range(NCH):
                            for hp in (0, 1):
                                hh = jt * 2 + hp
                                nc.tensor.matmul(
                                    ctx_ps[hp * 64:(hp + 1) * 64, :],
                                    v_sb[:, kc * JW + hh * DH: kc * JW + (hh + 1) * DH],
                                    pmTs[hp][:, kc * 128:(kc + 1) * 128],
                                    start=(kc == 0), stop=(kc == NCH - 1),
                                    tile_position=(0, hp * 64))
                        nc.scalar.copy(ctxT_sb[jt][:, it * 128:(it + 1) * 128], ctx_ps[:])

                    # ---- output projection for this l-tile ----
                    o_sb = fop.tile([128, DM], fp32, tag="osb", name="o_sb")
                    for mcb in range(2):
                        ps = wps.tile([128, 512], fp32, tag="finp", name="finp")
                        for jt in range(2):
                            nc.tensor.matmul(
                                ps[:],
                                ctxT_sb[jt][:, it * 128:(it + 1) * 128],
                                wo_sb[jt][:, mcb * 512:(mcb + 1) * 512],
                                start=(jt == 0), stop=(jt == 1))
                        nc.scalar.copy(o_sb[:, mcb * 512:(mcb + 1) * 512], ps[:])
                    nc.sync.dma_start(out[it * 128:(it + 1) * 128, :], o_sb[:])

    nc.compile()
    return nc


def _get_nc():
    if "nc" not in _CACHED:
        _CACHED["nc"] = _build_nc()
    return _CACHED["nc"]


def kernel(x, Wq, bq, Wk, bk, Wv, bv, Wo, bo, _trace=False, _tmpdir=None):
    from concourse.bass_utils import run_bass_kernel_spmd

    x = np.asarray(x, dtype=np.float32)
    Wq = np.asarray(Wq, dtype=np.float32)
    Wk = np.asarray(Wk, dtype=np.float32)
    Wv = np.asarray(Wv, dtype=np.float32)
    Wo = np.asarray(Wo, dtype=np.float32)
    bq = np.asarray(bq, dtype=np.float32)
    bk = np.asarray(bk, dtype=np.float32)
    bv = np.asarray(bv, dtype=np.float32)
    bo = np.asarray(bo, dtype=np.float32)

    in_maps = []
    for c in range(NCORES):
        b, g = c // 4, c % 4
        cols = slice(g * HPC * DH, (g + 1) * HPC * DH)
        in_maps.append({
            "xT": np.ascontiguousarray(x[b].T),
            "wq": np.ascontiguousarray(Wq.T[:, cols]),
            "wk": np.ascontiguousarray(Wk.T[:, cols]),
            "wv": np.ascontiguousarray(Wv.T[:, cols]),
            "wo": np.ascontiguousarray(Wo[:, cols].T),
            "bq": np.ascontiguousarray(bq[cols].reshape(-1, 1)),
            "bk": np.ascontiguousarray(bk[cols].reshape(-1, 1)),
        })

    nc = _get_nc()
    kw = {}
    if _trace:
        kw = dict(trace=True, tmpdir=_tmpdir, trace_cores=[0], stitch_traces=False)
    res = run_bass_kernel_spmd(nc, in_maps, core_ids=list(range(NCORES)), **kw)
    if _trace:
        _CACHED["last_results"] = res

    # host-side unshard: sum partials per batch + constant bias corrections
    corr = (bv @ Wo.T + bo).astype(np.float32)
    outp = np.empty((B, L, DM), dtype=np.float32)
    for b in range(B):
        acc = res.results[4 * b]["out"].astype(np.float32).copy()
        for g in range(1, 4):
            acc += res.results[4 * b + g]["out"]
        outp[b] = acc + corr
    return outp


# revision 18
# speedup vs baseline: 1.0424x; 1.0424x over previous
"""ProbSparse self-attention (Informer-style) TRN2 Bass kernel.

Full inputs in, full output out. Sharding: 8 cores = (2 batches) x (4 head
groups of 4 heads). Each core computes Q/K/V projections for its 4 heads,
exact top-38 sparse attention, and a partial output projection; the host sums
the 4 partials per batch and adds the (constant) bias corrections.

Per (head, q-tile) pipeline on device:
  PE:   S = Q_h q-tile @ K_h^T            (4x [128,512] psum matmuls, K=64)
  ACT:  s_sb = copy(S_psum)
  DVE:  exact top-38 threshold per row via chunked max8/match_replace:
        16x per-chunk top-8 -> candidates; leftovers R; top-8(R) supplement
        (covers up to 8 "hidden" values; P(fail) ~1e-8/row); 5 rounds of
        max8/match_replace over the 136 candidates -> sorted top-40 (mvals)
  ACT:  p = exp(s/8); Z = sum(exp(top38/8)) with exact-tie handling
  GPS:  mask = (s >= thr) * (1/Z); pm = mask * p          (normalized probs)
  PE:   16x transpose(pm chunk) -> psum (4 per bank); ACT copies -> pmT
  PE:   ctx^T = sum_k V_chunk-as-lhsT @ pmT chunk, two heads col-tiled into
        one [128,128] psum bank (tile_position concurrency)
Then out_partial = ctx^T.T @ Wo_slice^T via PE per l-tile (inline), DMA'd out.
"""

import numpy as np

B = 2
L = 2048
DM = 1024
H = 16
DH = 64          # head dim
HPC = 4          # heads per core
U = 38           # top-k (min(L, int(5*log(2048))))
NCH = 16         # key chunks of 128 per row
NEG = -3.0e38
NCORES = 8

_CACHED = {}


def _build_nc():
    import concourse.mybir as mybir
    import concourse.tile as tile
    from concourse import bacc
    from concourse.masks import make_identity

    fp32 = mybir.dt.float32
    Alu = mybir.AluOpType
    Act = mybir.ActivationFunctionType

    nc = bacc.Bacc("TRN2", target_bir_lowering=False, debug=False)

    xT = nc.dram_tensor("xT", [DM, L], fp32, kind="ExternalInput")
    wq = nc.dram_tensor("wq", [DM, HPC * DH], fp32, kind="ExternalInput")
    wk = nc.dram_tensor("wk", [DM, HPC * DH], fp32, kind="ExternalInput")
    wv = nc.dram_tensor("wv", [DM, HPC * DH], fp32, kind="ExternalInput")
    wo = nc.dram_tensor("wo", [HPC * DH, DM], fp32, kind="ExternalInput")
    bq = nc.dram_tensor("bq", [HPC * DH, 1], fp32, kind="ExternalInput")
    bk = nc.dram_tensor("bk", [HPC * DH, 1], fp32, kind="ExternalInput")
    out = nc.dram_tensor("out", [L, DM], fp32, kind="ExternalOutput")

    JW = HPC * DH          # 256 local head-dim width
    NQT = L // 128         # 16 q tiles
    with tile.TileContext(nc) as tc:
        with tc.tile_pool(name="persist", bufs=1) as pp:
            # Persistent SBUF tensors
            qt_sb = [pp.tile([128, L], fp32, tag=f"qt{j}", name=f"qt{j}") for j in range(2)]
            kt_sb = [pp.tile([128, L], fp32, tag=f"kt{j}", name=f"kt{j}") for j in range(2)]
            v_sb = pp.tile([128, NQT * JW], fp32, tag="v", name="v_sb")      # [l-chunk, lt*256 + hh*64 + d]
            ctxT_sb = [pp.tile([128, L], fp32, tag=f"ctxT{j}", name=f"ctxT{j}") for j in range(2)]
            wo_sb = [pp.tile([128, DM], fp32, tag=f"wo{j}", name=f"wo{j}") for j in range(2)]
            ident = pp.tile([128, 128], fp32, tag="ident", name="ident")
            bq_sb = [pp.tile([128, 1], fp32, tag=f"bq{j}", name=f"bq{j}") for j in range(2)]
            bk_sb = [pp.tile([128, 1], fp32, tag=f"bk{j}", name=f"bk{j}") for j in range(2)]

            make_identity(nc, ident[:])
            for j in range(2):
                nc.sync.dma_start(wo_sb[j][:], wo[j * 128:(j + 1) * 128, :])
                nc.sync.dma_start(bq_sb[j][:], bq[j * 128:(j + 1) * 128, :])
                nc.sync.dma_start(bk_sb[j][:], bk[j * 128:(j + 1) * 128, :])

            # ---------------- Projections ----------------
            with tc.tile_pool(name="proj", bufs=1) as xp, \
                 tc.tile_pool(name="projw", bufs=1) as wp, \
                 tc.tile_pool(name="projps", bufs=4, space="PSUM") as pps:
                xt_t = [xp.tile([128, L], fp32, tag=f"xt{cc}", name=f"xt{cc}") for cc in range(8)]
                wq_t = wp.tile([128, 8 * JW], fp32, tag="wqt", name="wq_t")
                wk_t = wp.tile([128, 8 * JW], fp32, tag="wkt", name="wk_t")
                wv_t = wp.tile([128, 8 * JW], fp32, tag="wvt", name="wv_t")
                for cc in range(8):
                    nc.sync.dma_start(xt_t[cc][:], xT[cc * 128:(cc + 1) * 128, :])
                    nc.sync.dma_start(wq_t[:, cc * JW:(cc + 1) * JW], wq[cc * 128:(cc + 1) * 128, :])
                    nc.sync.dma_start(wk_t[:, cc * JW:(cc + 1) * JW], wk[cc * 128:(cc + 1) * 128, :])
                    nc.sync.dma_start(wv_t[:, cc * JW:(cc + 1) * JW], wv[cc * 128:(cc + 1) * 128, :])

                # Q^T, K^T: [256 j, 2048 l] as 2 tiles of [128, 2048]
                for (wt, dst, bias) in ((wq_t, qt_sb, bq_sb), (wk_t, kt_sb, bk_sb)):
                    for jt in range(2):
                        for lc in range(4):
                            ps = pps.tile([128, 512], fp32, tag="projp", name="projp")
                            for cc in range(8):
                                nc.tensor.matmul(
                                    ps[:],
                                    wt[:, cc * JW + jt * 128: cc * JW + (jt + 1) * 128],
                                    xt_t[cc][:, lc * 512:(lc + 1) * 512],
                                    start=(cc == 0), stop=(cc == 7))
                            nc.vector.tensor_scalar(
                                out=dst[jt][:, lc * 512:(lc + 1) * 512], in0=ps[:],
                                scalar1=bias[jt][:, 0:1], scalar2=None, op0=Alu.add)
                # V: [2048 l, 256 j] as v_sb[l%128, (l//128)*256 + j]
                for lt in range(NQT):
                    ps = pps.tile([128, JW], fp32, tag="projv", name="projv")
                    for cc in range(8):
                        nc.tensor.matmul(
                            ps[:],
                            xt_t[cc][:, lt * 128:(lt + 1) * 128],
                            wv_t[:, cc * JW:(cc + 1) * JW],
                            start=(cc == 0), stop=(cc == 7))
                    nc.scalar.copy(v_sb[:, lt * JW:(lt + 1) * JW], ps[:])

            # ---------------- Attention (it-major; Wo inline per l-tile) ----------------
            with tc.tile_pool(name="att", bufs=2) as ap, \
                 tc.tile_pool(name="atts", bufs=4) as sp, \
                 tc.tile_pool(name="fin", bufs=3) as fop, \
                 tc.tile_pool(name="attps", bufs=4, space="PSUM") as sps, \
                 tc.tile_pool(name="attpt", bufs=2, space="PSUM") as tps, \
                 tc.tile_pool(name="attpc", bufs=1, space="PSUM") as cps, \
                 tc.tile_pool(name="attpw", bufs=1, space="PSUM") as wps:

                def head_scores(jt, po, it):
                    """scores for one head -> s_sb (SBUF)"""
                    s_sb = ap.tile([128, L], fp32, tag="s_sb", name="s_sb", bufs=4)
                    for ncb in range(4):
                        s_ps = sps.tile([128, 512], fp32, tag="sps", name="s_ps")
                        nc.tensor.matmul(
                            s_ps[:],
                            qt_sb[jt][po:po + 64, it * 128:(it + 1) * 128],
                            kt_sb[jt][po:po + 64, ncb * 512:(ncb + 1) * 512],
                            start=True, stop=True)
                        nc.scalar.copy(s_sb[:, ncb * 512:(ncb + 1) * 512], s_ps[:])
                    return s_sb

                def head_softmax(s_sb):
                    """top-38 -> normalized masked probs -> pmT"""
                    # exact top-38 threshold
                    cand = sp.tile([128, NCH * 8 + 8], fp32, tag="cand", name="cand")
                    r = ap.tile([128, L], fp32, tag="r", name="r", bufs=3)
                    for c in range(NCH):
                        sl = slice(c * 128, (c + 1) * 128)
                        nc.vector.max(out=cand[:, c * 8:(c + 1) * 8], in_=s_sb[:, sl])
                        nc.vector.match_replace(
                            out=r[:, sl], in_to_replace=cand[:, c * 8:(c + 1) * 8],
                            in_values=s_sb[:, sl], imm_value=NEG)
                    nc.vector.max(out=cand[:, NCH * 8:], in_=r[:])
                    mvals = sp.tile([128, 40], fp32, tag="mvals", name="mvals")
                    scratch = sp.tile([128, NCH * 8 + 8], fp32, tag="scr", name="scratch")
                    cur = cand
                    for rnd in range(5):
                        nc.vector.max(out=mvals[:, rnd * 8:(rnd + 1) * 8], in_=cur[:])
                        if rnd < 4:
                            nc.vector.match_replace(
                                out=scratch[:], in_to_replace=mvals[:, rnd * 8:(rnd + 1) * 8],
                                in_values=cur[:], imm_value=NEG)
                            cur = scratch
                    thr = mvals[:, U - 1:U]

                    # Z = sum(exp(top-38/8)) with exact-tie handling
                    emv = sp.tile([128, 40], fp32, tag="emv", name="emv")
                    nc.scalar.activation(emv[:], mvals[:], Act.Exp, scale=0.125)

                    # unnormalized exp (independent of Z)
                    p = ap.tile([128, L], fp32, tag="p", name="p", bufs=3)
                    nc.scalar.activation(p[:], s_sb[:], Act.Exp, scale=0.125)
                    tie = sp.tile([128, 2], fp32, tag="tie", name="tie")
                    nc.vector.tensor_scalar(out=tie[:], in0=mvals[:, U:],
                                            scalar1=thr, scalar2=None, op0=Alu.is_ge)
                    nc.vector.tensor_tensor(out=emv[:, U:], in0=emv[:, U:],
                                            in1=tie[:], op=Alu.mult)
                    zz = sp.tile([128, 1], fp32, tag="zz", name="zz")
                    nc.vector.reduce_sum(zz[:], emv[:], axis=mybir.AxisListType.X)
                    rz = sp.tile([128, 1], fp32, tag="rz", name="rz")
                    nc.vector.reciprocal(rz[:], zz[:])

                    # masked, 1/Z-scaled probabilities (gpsimd)
                    mask = ap.tile([128, L], fp32, tag="r", name="mask", bufs=3)
                    nc.gpsimd.tensor_scalar(out=mask[:], in0=s_sb[:], scalar1=thr,
                                            scalar2=rz[:, 0:1], op0=Alu.is_ge,
                                            op1=Alu.mult)
                    pm = p
                    nc.gpsimd.tensor_tensor(out=pm[:], in0=mask[:], in1=p[:], op=Alu.mult)

                    # transpose pm (4 transposes per psum bank, 1 copy per bank)
                    pmT = ap.tile([128, L], fp32, tag="pmT", name="pmT", bufs=3)
                    for g in range(4):
                        pt_ps = tps.tile([128, 512], fp32, tag="ptps", name="pt_ps")
                        for cc in range(4):
                            c = g * 4 + cc
                            nc.tensor.transpose(pt_ps[:, cc * 128:(cc + 1) * 128],
                                                pm[:, c * 128:(c + 1) * 128], ident[:])
                        nc.scalar.copy(pmT[:, g * 512:(g + 1) * 512], pt_ps[:])
                    return pmT

                def emit_wo(lt):
                    # output projection for one finished l-tile
                    o_sb = fop.tile([128, DM], fp32, tag="osb", name="o_sb")
                    for mcb in range(2):
                        ps = wps.tile([128, 512], fp32, tag="finp", name="finp")
                        for jt in range(2):
                            nc.tensor.matmul(
                                ps[:],
                                ctxT_sb[jt][:, lt * 128:(lt + 1) * 128],
                                wo_sb[jt][:, mcb * 512:(mcb + 1) * 512],
                                start=(jt == 0), stop=(jt == 1))
                        nc.scalar.copy(o_sb[:, mcb * 512:(mcb + 1) * 512], ps[:])
                    nc.sync.dma_start(out[lt * 128:(lt + 1) * 128, :], o_sb[:])

                for it in range(NQT):
                    for jt in range(2):
                        # both heads' score matmuls adjacent in the PE stream:
                        # disjoint row-groups (0-63 / 64-127) overlap on HW
                        s_sbs = [head_scores(jt, hp * 64, it) for hp in (0, 1)]
                        if jt == 0 and it > 0:
                            # pipeline the previous l-tile's output projection
                            # behind the new scores so PE doesn't stall DVE
                            emit_wo(it - 1)
                        pmTs = [head_softmax(s_sbs[hp]) for hp in (0, 1)]
                        # paired AV: two heads col-tiled into one [128,128] psum bank
                        ctx_ps = cps.tile([128, 128], fp32, tag="ctxps", name="ctx_ps")
                        for kc in range(NCH):
                            for hp in (0, 1):
                                hh = jt * 2 + hp
                                nc.tensor.matmul(
                                    ctx_ps[hp * 64:(hp + 1) * 64, :],
                                    v_sb[:, kc * JW + hh * DH: kc * JW + (hh + 1) * DH],
                                    pmTs[hp][:, kc * 128:(kc + 1) * 128],
                                    start=(kc == 0), stop=(kc == NCH - 1),
                                    tile_position=(0, hp * 64))
                        nc.scalar.copy(ctxT_sb[jt][:, it * 128:(it + 1) * 128], ctx_ps[:])
                emit_wo(NQT - 1)

    nc.compile()
    return nc


def _get_nc():
    if "nc" not in _CACHED:
        _CACHED["nc"] = _build_nc()
    return _CACHED["nc"]


def kernel(x, Wq, bq, Wk, bk, Wv, bv, Wo, bo, _trace=False, _tmpdir=None):
    from concourse.bass_utils import run_bass_kernel_spmd

    x = np.asarray(x, dtype=np.float32)
    Wq = np.asarray(Wq, dtype=np.float32)
    Wk = np.asarray(Wk, dtype=np.float32)
    Wv = np.asarray(Wv, dtype=np.float32)
    Wo = np.asarray(Wo, dtype=np.float32)
    bq = np.asarray(bq, dtype=np.float32)
    bk = np.asarray(bk, dtype=np.float32)
    bv = np.asarray(bv, dtype=np.float32)
    bo = np.asarray(bo, dtype=np.float32)

    in_maps = []
    for c in range(NCORES):
        b, g = c // 4, c % 4
        cols = slice(g * HPC * DH, (g + 1) * HPC * DH)
        in_maps.append({
            "xT": np.ascontiguousarray(x[b].T),
            "wq": np.ascontiguousarray(Wq.T[:, cols]),
            "wk": np.ascontiguousarray(Wk.T[:, cols]),
            "wv": np.ascontiguousarray(Wv.T[:, cols]),
            "wo": np.ascontiguousarray(Wo[:, cols].T),
            "bq": np.ascontiguousarray(bq[cols].reshape(-1, 1)),
            "bk": np.ascontiguousarray(bk[cols].reshape(-1, 1)),
        })

    nc = _get_nc()
    kw = {}
    if _trace:
        kw = dict(trace=True, tmpdir=_tmpdir, trace_cores=[0], stitch_traces=False)
    res = run_bass_kernel_spmd(nc, in_maps, core_ids=list(range(NCORES)), **kw)
    if _trace:
        _CACHED["last_results"] = res

    # host-side unshard: sum partials per batch + constant bias corrections
    corr = (bv @ Wo.T + bo).astype(np.float32)
    outp = np.empty((B, L, DM), dtype=np.float32)
    for b in range(B):
        acc = res.results[4 * b]["out"].astype(np.float32).copy()
        for g in range(1, 4):
            acc += res.results[4 * b + g]["out"]
        outp[b] = acc + corr
    return outp
